# revision 20
# baseline (speedup 1.0000x reference)
"""GroupGRUCell with shared schema-pool parameters — Trainium2 Bass kernel.

Problem shapes (hardcoded): B=256 batch, U=64 GRU units, DIN=H=256, S=8 schemas.
  Wx[u] = sum_s sw_x[u,s] * pool_x[s].T   (per-unit weights from shared pool)
  gate_x = x @ Wx ; gate_h = h @ Wh ; standard GRU cell gate math.

Sharding: unit-parallel across 8 NeuronCores (8 units per core); the schema
pool is replicated per core.

v3 design:
  * pool and the combined weights W are float8 e3m4 scaled by 32 (W absmax
    ~0.24 -> ~7.6, inside e3m4 range; e2e rel-err ~1e-2 vs the 2e-2 gate).
    PE does mixed e3m4 x bf16 matmuls (verified bit-accurate on HW).
  * shuffle-free combine: the pool slice is the STATIONARY operand and a
    block-diagonal sw matrix streams, so W lands in PSUM already in the
    d%128-partition layout the gate matmuls consume:
      lhsT[(s,oc), dm] = 32 * pool[s, ob*16+oc, kc*128+dm]   (per (kc,ob))
      rhs [(s,oc), (u,oc')] = sw[u,s] * delta(oc,oc')        (constant)
      out [dm, (u,oc)] = 32 * W[u, kc*128+dm, ob*16+oc]
    K=128 fully used; 192 matmuls of 128 columns; the previous SBUF->SBUF
    partition-regroup shuffle (3.15MB of DMA) is gone entirely.
  * casts PSUM->e3m4 SBUF batch 8 matmuls (2 PSUM banks) per op and write
    through a rearranged AP into the gate weight layout; ACT/DVE alternate.
  * gate pre-activations carry the x32 factor in PSUM; sigmoid/tanh fold
    the 1/32 into their activation scale.
  * gate tail math in f32 (measured faster than bf16 on DVE): ACT does
    sigmoid/tanh, DVE does t1/t2/out, GPSIMD does d/e.
  * loads: sync HWDGE queue carries sw/pools/hbh + hy stores; scalar HWDGE
    carries xt/ht.  Nothing uses the slow gpsimd SWDGE path.
"""

import numpy as np
import ml_dtypes

B, U, DIN, H, S = 256, 64, 256, 256, 8
NCORES = 8
UC = U // NCORES  # units per core
O3 = 3 * H        # 768
KC = DIN // 128   # 2 contraction chunks
MC = B // 128     # 2 batch chunks
NOB = O3 // 16    # 48 o-blocks of 16
WSCALE = 32.0     # host-side pool scale folded out in the activations
SWSCALE = 16.0    # host-side sw scale folded out in the combine cast

BF16 = ml_dtypes.bfloat16
E3M4 = ml_dtypes.float8_e3m4


def _build_program():
    from contextlib import ExitStack

    import concourse.bacc as bacc
    import concourse.mybir as mybir
    import concourse.tile as tile

    bf = mybir.dt.bfloat16
    f32 = mybir.dt.float32
    e3 = mybir.dt.float8e3
    AF = mybir.ActivationFunctionType
    ALU = mybir.AluOpType

    nc = bacc.Bacc("TRN2", target_bir_lowering=False, debug=False)

    # pool in combine-lhsT layout: [(s,oc), ob, kc, dm]
    poolx = nc.dram_tensor("poolx", [128, NOB, KC, 128], e3, kind="ExternalInput")
    poolh = nc.dram_tensor("poolh", [128, NOB, KC, 128], e3, kind="ExternalInput")
    swx = nc.dram_tensor("swx", [128, 128], e3, kind="ExternalInput")
    swh = nc.dram_tensor("swh", [128, 128], e3, kind="ExternalInput")
    xt = nc.dram_tensor("xt", [128, UC, KC, B], bf, kind="ExternalInput")
    ht = nc.dram_tensor("ht", [128, UC, KC, B], bf, kind="ExternalInput")
    hbh = nc.dram_tensor("hbh", [128, UC, MC, H], f32, kind="ExternalInput")
    hy = nc.dram_tensor("hy", [128, UC, MC, H], bf, kind="ExternalOutput")

    NCH = 6           # pool DMA chunks per side
    OBC = NOB // NCH  # 8 o-blocks per chunk

    with tile.TileContext(nc) as tc, ExitStack() as ctx:
        pconst = ctx.enter_context(tc.tile_pool(name="pconst", bufs=1))
        pgtmp = ctx.enter_context(tc.tile_pool(name="pgtmp", bufs=4))

        # --- input loads ---
        # pool chunks split across both HWDGE queues: pool-x on sync (paces
        # the first combine sweep), pool-h on scalar.  xt/ht/hbh follow the
        # pool-x chunks on sync — gates don't need them until ~30us in.
        pool_c = {
            (t, c): pconst.tile(
                [128, OBC, KC, 128], e3, tag=f"pool{t}{c}", name=f"pool{t}{c}"
            )
            for t in ("x", "h")
            for c in range(NCH)
        }

        def load_pool(t, c, eng):
            dram = poolx if t == "x" else poolh
            eng.dma_start(
                out=pool_c[(t, c)], in_=dram[:, c * OBC : (c + 1) * OBC, :, :]
            )

        load_pool("x", 0, nc.sync)
        swx_sb = pconst.tile([128, 128], e3, tag="swx")
        nc.sync.dma_start(out=swx_sb, in_=swx[:, :])
        swh_sb = pconst.tile([128, 128], e3, tag="swh")
        nc.sync.dma_start(out=swh_sb, in_=swh[:, :])
        for c in range(NCH):
            load_pool("h", c, nc.scalar)
        for c in range(1, NCH):
            load_pool("x", c, nc.sync)
        xt_sb = pconst.tile([128, UC, KC, B], bf, tag="xt")
        nc.sync.dma_start(out=xt_sb, in_=xt[:, :, :, :])
        ht_sb = pconst.tile([128, UC, KC, B], bf, tag="ht")
        nc.sync.dma_start(out=ht_sb, in_=ht[:, :, :, :])
        # hbh as 4 separate per-u-pair tiles: tile-granular deps mean d(u)
        # only waits for its own pair's slice
        hbh_sb = {}
        for c in range(4):
            hbh_sb[c] = pconst.tile(
                [128, 2, MC, H], f32, tag=f"hbh{c}", name=f"hbh{c}"
            )
            nc.sync.dma_start(
                out=hbh_sb[c], in_=hbh[:, 2 * c : 2 * c + 2, :, :]
            )

        # warm the ACT sigmoid/tanh tables during startup so the table load
        # (2x ~1.3us) doesn't sit between the combine casts and the first
        # real sigmoid
        warm = pconst.tile([128, 2], f32, tag="warm")
        nc.scalar.activation(out=warm[:, 0:1], in_=warm[:, 0:1], func=AF.Sigmoid)
        nc.scalar.activation(out=warm[:, 1:2], in_=warm[:, 1:2], func=AF.Tanh)

        # all units' combined weights, gate-matmul layout: [d%128, u, d//128, o]
        wp = {
            "x": pconst.tile([128, UC, KC, O3], e3, tag="wpx", name="wpx"),
            "h": pconst.tile([128, UC, KC, O3], e3, tag="wph", name="wph"),
        }
        out_sb = {
            i: pconst.tile([128, 2, MC, H], bf, tag=f"out{i}", name=f"out{i}")
            for i in range(UC // 2)
        }

        # --- schema combine on the PE, output directly in gate layout ---
        GRP = 8  # matmuls per cast group (2 PSUM banks)
        SWINV = float(1.0 / SWSCALE)
        with tc.tile_pool(name="pcomb", bufs=4, space="PSUM") as pcomb:
            cast_rr = 0
            # chunk-outer, x/h interleaved: the two pool streams arrive on
            # separate queues concurrently, so PE consumes both as they land
            for obb in range(NOB // GRP):
                for t, sw_sb in (("x", swx_sb), ("h", swh_sb)):
                    for kc in range(KC):
                        ps = pcomb.tile([128, GRP, 128], f32, tag="cps")
                        for j in range(GRP):
                            ob = obb * GRP + j
                            nc.tensor.matmul(
                                ps[:, j, :],
                                pool_c[(t, ob // OBC)][:, ob % OBC, kc, :],
                                sw_sb,
                                start=True, stop=True,
                            )
                        # dst iterates (ob, u, oc) to match PSUM (j, (u,oc))
                        dst = wp[t][
                            :, :, kc, obb * GRP * 16 : (obb + 1) * GRP * 16
                        ].rearrange("p u (a b) -> p a u b", a=GRP)
                        if cast_rr % 2 == 0:
                            nc.scalar.activation(
                                out=dst, in_=ps, func=AF.Copy, scale=SWINV
                            )
                        else:
                            nc.vector.tensor_scalar_mul(
                                out=dst, in0=ps, scalar1=SWINV
                            )
                        cast_rr += 1

        # --- gate matmuls + GRU gate math (batched over both mc halves) ---
        # Elementwise work is software-pipelined with a one-unit skew so no
        # engine's in-order queue blocks on a cross-engine dependency:
        #   ACT: sig(u), tanh(u-1)   DVE: t1(u), t2(u), out(u-1)
        #   GPSIMD: d(u-1), e(u-1)   sync: hy store (u-1 pair)
        INV = float(1.0 / WSCALE)
        stage2 = {}  # u -> (sig, nxh-derived tiles) for the skewed back half

        def emit_front(u, pg):
            rib = pg.tile([128, MC, 512], f32, tag="ri", name="ri")
            nxb = pg.tile([128, MC, 512], f32, tag="nxh", name="nxh")
            for mc in range(MC):
                bs = slice(mc * 128, (mc + 1) * 128)
                for t, t_sb, nlo in (("x", xt_sb, 0), ("h", ht_sb, 256)):
                    for kc in range(KC):
                        lhs = t_sb[:, u, kc, bs]
                        nc.tensor.matmul(
                            rib[:, mc, :], lhs, wp[t][:, u, kc, 0:512],
                            start=(t == "x" and kc == 0),
                            stop=(t == "h" and kc == 1),
                        )
                        nc.tensor.matmul(
                            nxb[:, mc, nlo : nlo + 256],
                            lhs, wp[t][:, u, kc, 512:O3],
                            start=(kc == 0), stop=(kc == 1),
                        )
            # sig = [rg | ig] per mc; 1/32 folded into the ACT scale
            sig = pgtmp.tile([128, MC, 512], f32, tag="sig")
            nc.scalar.activation(out=sig, in_=rib, func=AF.Sigmoid, scale=INV)
            t1 = pgtmp.tile([128, MC, H], f32, tag="t1")
            nc.vector.tensor_tensor(
                out=t1, in0=sig[:, :, 0:H], in1=nxb[:, :, 256:512], op=ALU.mult
            )
            t2 = pgtmp.tile([128, MC, H], f32, tag="t2")
            nc.vector.tensor_tensor(
                out=t2, in0=t1, in1=nxb[:, :, 0:256], op=ALU.add
            )
            stage2[u] = (sig, t2)

        def emit_back(u):
            sig, t2 = stage2.pop(u)
            # t2 still carries x32; fold 1/32 into the tanh scale
            ng = pgtmp.tile([128, MC, H], f32, tag="ng")
            nc.scalar.activation(out=ng, in_=t2, func=AF.Tanh, scale=INV)
            # d on GPSIMD (except last unit: DVE, to shorten the drain
            # chain); e alternates DVE/GPSIMD to balance engine load
            deng = nc.vector if u == UC - 1 else nc.gpsimd
            eeng = nc.vector if (u % 2 == 1 or u == UC - 1) else nc.gpsimd
            d = pgtmp.tile([128, MC, H], f32, tag="d")
            deng.tensor_tensor(
                out=d, in0=hbh_sb[u // 2][:, u % 2, :, :], in1=ng, op=ALU.subtract
            )
            e = pgtmp.tile([128, MC, H], f32, tag="e")
            eeng.tensor_tensor(
                out=e, in0=sig[:, :, 256:512], in1=d, op=ALU.mult
            )
            nc.vector.tensor_tensor(
                out=out_sb[u // 2][:, u % 2, :, :], in0=ng, in1=e, op=ALU.add
            )
            if u % 2 == 1:
                nc.sync.dma_start(
                    out=hy[:, u - 1 : u + 1, :, :],
                    in_=out_sb[u // 2][:, :, :, :],
                )

        with tc.tile_pool(name="pg", bufs=2, space="PSUM") as pg:
            for u in range(UC):
                emit_front(u, pg)
                if u >= 1:
                    emit_back(u - 1)
            emit_back(UC - 1)

    nc.compile()
    return nc


def _prep_inputs(x, hidden, pool_x, pool_h, sw_x, sw_h):
    """Host-side (free) slicing / transposition / casting per core."""
    # pool[s, o, d] -> lhsT layout [(s,oc), ob, kc, dm], o = ob*16+oc, d = kc*128+dm
    def prep_pool(p):
        pt = (p * WSCALE).reshape(S, NOB, 16, KC, 128)  # [s, ob, oc, kc, dm]
        pt = pt.transpose(0, 2, 1, 3, 4)                # [s, oc, ob, kc, dm]
        return np.ascontiguousarray(pt.reshape(128, NOB, KC, 128).astype(E3M4))

    poolx_h = prep_pool(pool_x)
    poolh_h = prep_pool(pool_h)

    in_maps = []
    for c in range(NCORES):
        us = slice(c * UC, (c + 1) * UC)

        def sw_block(sw_c):  # [UC, S] -> [(s,oc), (u,oc')] block-diagonal
            blk = np.zeros((S, 16, UC, 16), dtype=np.float32)
            for oc in range(16):
                blk[:, oc, :, oc] = sw_c.T * SWSCALE
            return np.ascontiguousarray(blk.reshape(128, 128).astype(E3M4))

        xc = x[:, us, :]       # [B, UC, DIN]
        hc = hidden[:, us, :]
        # [128 (d%128), UC, KC (d//128), B]
        xt_h = np.ascontiguousarray(
            xc.transpose(1, 2, 0).reshape(UC, KC, 128, B).transpose(2, 0, 1, 3).astype(BF16)
        )
        ht_h = np.ascontiguousarray(
            hc.transpose(1, 2, 0).reshape(UC, KC, 128, B).transpose(2, 0, 1, 3).astype(BF16)
        )
        # [128 (b%128), UC, MC (b//128), H]
        hbh_h = np.ascontiguousarray(
            hc.reshape(MC, 128, UC, H).transpose(1, 2, 0, 3).astype(np.float32)
        )
        in_maps.append(
            {
                "poolx": poolx_h,
                "poolh": poolh_h,
                "swx": sw_block(sw_x[us]),
                "swh": sw_block(sw_h[us]),
                "xt": xt_h,
                "ht": ht_h,
                "hbh": hbh_h,
            }
        )
    return in_maps


_CACHED_NC = None


def _get_nc():
    global _CACHED_NC
    if _CACHED_NC is None:
        _CACHED_NC = _build_program()
    return _CACHED_NC


def kernel(x, hidden, pool_x, pool_h, sw_x, sw_h, _trace=False, _results_holder=None):
    from concourse.bass_utils import run_bass_kernel_spmd

    x = np.asarray(x)
    hidden = np.asarray(hidden)
    pool_x = np.asarray(pool_x)
    pool_h = np.asarray(pool_h)
    sw_x = np.asarray(sw_x)
    sw_h = np.asarray(sw_h)

    nc = _get_nc()
    in_maps = _prep_inputs(x, hidden, pool_x, pool_h, sw_x, sw_h)
    res = run_bass_kernel_spmd(
        nc, in_maps, core_ids=list(range(NCORES)), trace=_trace
    )
    if _results_holder is not None:
        _results_holder.append(res)

    out = np.empty((B, U, H), dtype=np.float32)
    for c in range(NCORES):
        hy_c = np.asarray(res.results[c]["hy"]).astype(np.float32)  # [128, UC, MC, H]
        # out[b, u, h] with b = mc*128 + bp
        out[:, c * UC : (c + 1) * UC, :] = hy_c.transpose(2, 0, 1, 3).reshape(B, UC, H)
    return out
